# revision 45
# baseline (speedup 1.0000x reference)
"""Trainium2 Bass kernel for BGNN-A message passing (nn_BGNNA_33767032881163).

Math (reference):
    adj  = edge + I                       (edge entries are exactly 0/1)
    out  = norm * ((adj @ xw)^2 - adj^2 @ xw^2) + bias
    norm = 1 / (rowsum(adj)^2 - rowsum(adj^2)),  inf -> 0
    xw   = x @ weight

Kernel formulation (exploits binarity of edge: edge^2 == edge and
adj^2 == edge + diag(2*d + 1) with d = diag(edge)):
    P   = edge_rows @ B,  B = [1 | xw | xw^2]   (N x 65)  <- ONE fused matmul
    r   = P[:,0]                                 (edge row sums)
    s   = P[:,1:33] + xw_rows                    (adj @ xw)
    q   = P[:,33:65]                             (edge @ xw^2)
    den = r^2 + r - 2*d                          (exact integers in f32)
    out = nrm * (s^2 - q - (2*d+1)*xw2_rows) + bias,  nrm = den/(den^2+eps)

Distribution: 1D row shard of edge across 8 cores (1536 rows each); B/xw is
computed on every core from the replicated x (x is tiny).  The edge shard is
cast on the host to a 2-byte-transposable format (lossless for 0/1 values)
and streamed transposed into the PE with HWDGE DMA-transpose; no on-chip
transposition of the big matrix is needed.

Variants:
  fp8dr (default): edge as fp8 pairs packed in u16; moving operand feeds the
      PE in DoubleRow mode (256 contraction rows per matmul, 0.5 cyc/row).
      B is decomposed into 4 fp8 components (comp2/3 pre-scaled by 2^8 and
      accumulated in a second PSUM region merged with 2^-8 at the epilogue),
      giving ~16-17 effective mantissa bits.
  bf16pair: edge as bf16; B as bf16 hi+lo pair (two matmuls per strip).

Known pitfall encoded here: ALL transpose DMAs are issued on a single HWDGE
queue (nc.sync).  Alternating sync/scalar queues produced nondeterministic
corruption (out-of-order completion vs semaphore accounting).
"""

import numpy as np
import ml_dtypes

N_NODES = 12288
IN_CH = 64
OUT_CH = 32
N_CORES = 8
P = 128  # partitions

VARIANT = "fp8dr"

_BUILD_CACHE = {}


def _build(n_nodes: int, n_cores: int, variant: str = VARIANT):
    import concourse.mybir as mybir
    import concourse.tile as tile
    from concourse import bacc
    from contextlib import ExitStack

    f32 = mybir.dt.float32
    bf16 = mybir.dt.bfloat16
    fp8 = mybir.dt.float8e4
    u16 = mybir.dt.uint16

    rpc = n_nodes // n_cores          # rows per core
    nt = rpc // P                     # 128-row tiles per core
    ns = n_nodes // P                 # 128-col strips
    ns2 = ns // 2                     # 256-col double strips (fp8dr)
    ng = rpc // 512                   # moving-dim groups of 512
    ch = 2 * OUT_CH + 1               # B columns: [1 | xw | xw2]
    PL = 80                           # fp8dr plane pitch (step % 16 == 0)
    assert rpc % 512 == 0 and ns % 16 == 0 and nt * OUT_CH * 4 <= 2048

    nc = bacc.Bacc(
        "TRN2",
        target_bir_lowering=False,
        debug=False,
        enable_asserts=False,
        num_devices=n_cores,
    )

    if variant == "fp8dr":
        # fp8 edge packed as u16 pairs (two adjacent columns per element)
        edge_d = nc.dram_tensor(
            "edge", [rpc, n_nodes // 2], u16, kind="ExternalInput"
        ).ap()
    else:
        edge_d = nc.dram_tensor(
            "edge", [rpc, n_nodes], bf16, kind="ExternalInput"
        ).ap()
    xfull_d = nc.dram_tensor("xfull", [n_nodes, IN_CH], f32, kind="ExternalInput").ap()
    xrows_d = nc.dram_tensor("x_rows", [rpc, IN_CH], f32, kind="ExternalInput").ap()
    weight_d = nc.dram_tensor("weight", [IN_CH, OUT_CH], f32, kind="ExternalInput").ap()
    bias_d = nc.dram_tensor("bias_rep", [P, OUT_CH], f32, kind="ExternalInput").ap()
    diag_d = nc.dram_tensor("diag", [rpc], f32, kind="ExternalInput").ap()
    ident_d = nc.dram_tensor("ident", [P, P], f32, kind="ExternalInput").ap()
    out_d = nc.dram_tensor("out", [rpc, OUT_CH], f32, kind="ExternalOutput").ap()

    with tile.TileContext(nc) as tc, ExitStack() as ctx:
        konst = ctx.enter_context(tc.tile_pool(name="konst", bufs=1))
        ident = konst.tile([P, P], f32)
        nc.gpsimd.dma_start(ident, ident_d)
        weight_sb = konst.tile([IN_CH, OUT_CH], f32)
        nc.gpsimd.dma_start(weight_sb, weight_d)
        bias_sb = konst.tile([P, OUT_CH], f32)
        nc.gpsimd.dma_start(bias_sb, bias_d)
        diag_sb = konst.tile([P, nt], f32)
        nc.gpsimd.dma_start(diag_sb, diag_d.rearrange("(t p) -> p t", p=P))

        bthi = btlo = bthi3 = btlo3 = None
        comps4 = None
        if variant == "fp8dr":
            # 4 fp8 components of B; comp2/3 hold values * 2^8.  Layout per
            # 256-row double-strip: 2 planes (even/odd rows) of PL columns:
            # [1 | xw(32) | xw2(32) | pad].
            comps = [
                konst.tile([P, ns2 * 2 * PL], fp8, name=f"comp{k}")
                for k in range(4)
            ]
            comps4 = [
                c.rearrange("p (s pl c) -> p s pl c", pl=2, c=PL) for c in comps
            ]
        else:
            bthi = konst.tile([P, ns * ch], bf16)
            btlo = konst.tile([P, ns * ch], bf16)
            bthi3 = bthi.rearrange("p (s c) -> p s c", c=ch)
            btlo3 = btlo.rearrange("p (s c) -> p s c", c=ch)
        xw_nat = konst.tile([P, nt * OUT_CH], f32)
        xw2_nat = konst.tile([P, nt * OUT_CH], f32)
        xw_nat3 = xw_nat.rearrange("p (t c) -> p t c", c=OUT_CH)
        xw2_nat3 = xw2_nat.rearrange("p (t c) -> p t c", c=OUT_CH)

        # ---------------- stage 1: xw / B preparation ----------------
        with tc.tile_pool(name="s1", bufs=1) as s1, \
             tc.tile_pool(name="s1p", bufs=2, space="PSUM") as s1p, \
             tc.tile_pool(name="s1s", bufs=2) as s1s:
            xf_sb = s1.tile([P, ns * IN_CH], f32)
            xf_chunk = max(ns // 4, 1)
            for c0 in range(0, ns, xf_chunk):
                c1 = min(c0 + xf_chunk, ns)
                nc.gpsimd.dma_start(
                    xf_sb.rearrange("p (s k) -> p s k", k=IN_CH)[:, c0:c1, :],
                    xfull_d.rearrange("(s p) k -> p s k", p=P)[:, c0:c1, :],
                )
            xr_sb = s1.tile([P, nt * IN_CH], f32)
            nc.gpsimd.dma_start(
                xr_sb.rearrange("p (t k) -> p t k", k=IN_CH),
                xrows_d.rearrange("(t p) k -> p t k", p=P),
            )
            xf3 = xf_sb.rearrange("p (s k) -> p s k", k=IN_CH)
            xr3 = xr_sb.rearrange("p (t k) -> p t k", k=IN_CH)

            xT = s1.tile([IN_CH, n_nodes], f32)
            xTr = s1.tile([IN_CH, rpc], f32)

            # x^T via PE transpose, 4 tiles per PSUM bank
            for s in range(ns):
                if s % 4 == 0:
                    pt = s1p.tile([IN_CH, 512], f32, tag="pt")
                nc.tensor.transpose(
                    pt[:, (s % 4) * P:(s % 4 + 1) * P], xf3[:, s, :], ident
                )
                if s % 4 == 3:
                    nc.scalar.copy(xT[:, (s - 3) * P:(s + 1) * P], pt)
            for t in range(nt):
                if t % 4 == 0:
                    ptr = s1p.tile([IN_CH, 512], f32, tag="pt")
                nc.tensor.transpose(
                    ptr[:, (t % 4) * P:(t % 4 + 1) * P], xr3[:, t, :], ident
                )
                if t % 4 == 3:
                    nc.scalar.copy(xTr[:, (t - 3) * P:(t + 1) * P], ptr)

            # xw strips (natural row-major layout) + split into B components
            if variant == "fp8dr":
                # pair-interleaved column view of x^T: plane i = rows 2j+i
                xTv = xT.rearrange("k (j2 two) -> k two j2", two=2)
            for s in range(ns):
                if s % 16 == 0:
                    pw = s1p.tile([P, 16 * OUT_CH], f32, tag="pw")
                if variant == "fp8dr":
                    s2, par = s // 2, s % 2
                    lhs_s = xTv[:, par, s2 * P:(s2 + 1) * P]
                else:
                    lhs_s = xT[:, s * P:(s + 1) * P]
                nc.tensor.matmul(
                    pw[:, (s % 16) * OUT_CH:(s % 16 + 1) * OUT_CH],
                    lhsT=lhs_s,
                    rhs=weight_sb,
                    start=True,
                    stop=True,
                )
                if s % 16 != 15:
                    continue
                sq = s1s.tile([P, 16 * OUT_CH], f32, tag="sq")
                nc.scalar.activation(sq, pw, mybir.ActivationFunctionType.Square)
                if variant == "fp8dr":
                    # batch of 8 double-strips: psum slots are [s2-rel, par, c]
                    b0 = (s - 15) // 2
                    pw4 = pw.rearrange("p (s2 pl c) -> p s2 pl c", pl=2, c=OUT_CH)
                    sq4 = sq.rearrange("p (s2 pl c) -> p s2 pl c", pl=2, c=OUT_CH)

                    def dsts(k, lo):
                        return comps4[k][:, b0:b0 + 8, :, lo:lo + OUT_CH]

                    for src4, lo in ((pw4, 1), (sq4, 1 + OUT_CH)):
                        tg = "a" if lo == 1 else "b"
                        cf = s1s.tile([P, 16 * OUT_CH], f32, tag="cf" + tg,
                                      name="cf")
                        cf4 = cf.rearrange(
                            "p (s2 pl c) -> p s2 pl c", pl=2, c=OUT_CH
                        )
                        r1 = s1s.tile([P, 16 * OUT_CH], f32, tag="r1" + tg,
                                      name="r1")
                        r14 = r1.rearrange(
                            "p (s2 pl c) -> p s2 pl c", pl=2, c=OUT_CH
                        )
                        r2 = s1s.tile([P, 16 * OUT_CH], f32, tag="r2" + tg,
                                      name="r2")
                        r24 = r2.rearrange(
                            "p (s2 pl c) -> p s2 pl c", pl=2, c=OUT_CH
                        )
                        # successive fp8 quantization; mixed-dtype TT inputs
                        # are avoided via explicit f32 copy-backs (cf).
                        nc.vector.tensor_copy(dsts(0, lo), src4)
                        nc.gpsimd.tensor_copy(cf4, dsts(0, lo))
                        nc.vector.tensor_sub(r14, src4, cf4)
                        nc.scalar.copy(dsts(1, lo), r14)
                        nc.gpsimd.tensor_copy(cf4, dsts(1, lo))
                        nc.vector.tensor_sub(r24, r14, cf4)
                        nc.vector.tensor_scalar_mul(dsts(2, lo), r24, 256.0)
                        nc.gpsimd.tensor_scalar_mul(cf4, dsts(2, lo), 1.0 / 256.0)
                        nc.vector.tensor_sub(r14, r24, cf4)
                        nc.scalar.activation(
                            dsts(3, lo), r14,
                            mybir.ActivationFunctionType.Copy, scale=256.0,
                        )
                else:
                    g0 = s - 15
                    pw3 = pw.rearrange("p (s c) -> p s c", c=OUT_CH)
                    sq3 = sq.rearrange("p (s c) -> p s c", c=OUT_CH)
                    hi = bthi3[:, g0:s + 1, 1:1 + OUT_CH]
                    lo_ = btlo3[:, g0:s + 1, 1:1 + OUT_CH]
                    nc.vector.tensor_copy(hi, pw3)
                    nc.vector.tensor_sub(lo_, pw3, hi)
                    hi2 = bthi3[:, g0:s + 1, 1 + OUT_CH:ch]
                    lo2 = btlo3[:, g0:s + 1, 1 + OUT_CH:ch]
                    nc.vector.tensor_copy(hi2, sq3)
                    nc.vector.tensor_sub(lo2, sq3, hi2)

            # ones columns of B
            if variant == "fp8dr":
                nc.gpsimd.memset(comps4[0][:, :, :, 0:1], 1.0)
                for k in range(1, 4):
                    nc.gpsimd.memset(comps4[k][:, :, :, 0:1], 0.0)
            else:
                nc.gpsimd.memset(bthi3[:, :, 0:1], 1.0)
                nc.gpsimd.memset(btlo3[:, :, 0:1], 0.0)

            # xw for this core's own rows (natural layout) for the epilogue
            pn = s1p.tile([P, nt * OUT_CH], f32, tag="pn", bufs=1)
            for t in range(nt):
                nc.tensor.matmul(
                    pn[:, t * OUT_CH:(t + 1) * OUT_CH],
                    lhsT=xTr[:, t * P:(t + 1) * P],
                    rhs=weight_sb,
                    start=True,
                    stop=True,
                )
            nc.vector.tensor_copy(xw_nat, pn)
            nc.vector.tensor_mul(xw2_nat, xw_nat, xw_nat)

        # ---------------- stage 2: P = edge_rows @ B ----------------
        pmain = ctx.enter_context(tc.tile_pool(name="pmain", bufs=1, space="PSUM"))
        strips = ctx.enter_context(tc.tile_pool(name="strips", bufs=22))
        if variant == "fp8dr":
            p_psA = [
                pmain.tile([ch, 512], f32, tag=f"PA{g}", name=f"p_psA{g}")
                for g in range(ng)
            ]
            p_psB = [
                pmain.tile([ch, 512], f32, tag=f"PB{g}", name=f"p_psB{g}")
                for g in range(ng)
            ]
            for s2 in range(ns2):
                strip = strips.tile([P, rpc], u16, tag="strip")
                nc.sync.dma_start(
                    strip, edge_d[:, s2 * P:(s2 + 1) * P], transpose=True
                )
                sf8 = strip.bitcast(fp8).rearrange("p (r two) -> p two r", two=2)
                for k in range(4):
                    lhs = comps4[k][:, s2, :, 0:ch]
                    reg = p_psA if k < 2 else p_psB
                    for g in range(ng):
                        nc.tensor.matmul(
                            reg[g],
                            lhsT=lhs,
                            rhs=sf8[:, :, g * 512:(g + 1) * 512],
                            perf_mode=mybir.MatmulPerfMode.DoubleRow,
                            start=(s2 == 0 and k % 2 == 0),
                            stop=(s2 == ns2 - 1 and k % 2 == 1),
                        )
        else:
            p_ps = [
                pmain.tile([ch, 512], f32, tag=f"P{g}", name=f"p_ps{g}")
                for g in range(ng)
            ]
            for s in range(ns):
                strip = strips.tile([P, rpc], bf16, tag="strip")
                nc.sync.dma_start(
                    strip, edge_d[:, s * P:(s + 1) * P], transpose=True
                )
                # weight-grouped order: one LDW per component per strip
                for ci, comp in enumerate((bthi, btlo)):
                    for g in range(ng):
                        nc.tensor.matmul(
                            p_ps[g],
                            lhsT=comp[:, s * ch:(s + 1) * ch],
                            rhs=strip[:, g * 512:(g + 1) * 512],
                            start=(s == 0 and ci == 0),
                            stop=(s == ns - 1 and ci == 1),
                        )

        # ---------------- stage 3: epilogue ----------------
        with tc.tile_pool(name="epi", bufs=1) as ep, \
             tc.tile_pool(name="epip", bufs=2, space="PSUM") as epp:
            p_sb = ep.tile([ch, rpc], f32)
            if variant == "fp8dr":
                for g in range(ng):
                    dst = p_sb[:, g * 512:(g + 1) * 512]
                    nc.vector.tensor_copy(dst, p_psA[g])
                    tmb = ep.tile([ch, 512], f32, tag="tmb", name="tmb")
                    nc.scalar.activation(
                        tmb, p_psB[g],
                        mybir.ActivationFunctionType.Copy, scale=1.0 / 256.0,
                    )
                    nc.vector.tensor_add(dst, dst, tmb)
            else:
                for g in range(ng):
                    nc.vector.tensor_copy(p_sb[:, g * 512:(g + 1) * 512], p_ps[g])

            epi = ep.tile([P, nt * ch], f32)
            epi3 = epi.rearrange("p (t c) -> p t c", c=ch)
            for t in range(nt):
                pe_t = epp.tile([P, ch], f32, tag="pe")
                nc.tensor.transpose(pe_t, p_sb[:, t * P:(t + 1) * P], ident[:ch, :ch])
                nc.scalar.copy(epi3[:, t, :], pe_t)

            r = epi3[:, :, 0]                     # [P, nt] edge row sums
            den = ep.tile([P, nt], f32)
            d2 = ep.tile([P, nt], f32)
            nrm = ep.tile([P, nt], f32)
            nc.vector.tensor_mul(den, r, r)
            nc.vector.tensor_add(den, den, r)      # r^2 + r
            nc.vector.tensor_scalar_mul(d2, diag_sb, 2.0)
            nc.vector.tensor_sub(den, den, d2)     # r^2 + r - 2d  (exact)
            nc.vector.tensor_scalar_add(d2, d2, 1.0)  # 2d + 1
            nc.vector.tensor_mul(nrm, den, den)
            nc.vector.tensor_scalar_add(nrm, nrm, 1e-20)
            nc.vector.reciprocal(nrm, nrm)
            nc.vector.tensor_mul(nrm, nrm, den)    # den/(den^2+eps); 0 -> 0

            sf = ep.tile([P, nt * OUT_CH], f32)
            sf3 = sf.rearrange("p (t c) -> p t c", c=OUT_CH)
            nc.vector.tensor_add(sf3, epi3[:, :, 1:1 + OUT_CH], xw_nat3)
            aa = ep.tile([P, nt * OUT_CH], f32)
            aa3 = aa.rearrange("p (t c) -> p t c", c=OUT_CH)
            nc.vector.tensor_mul(aa3, sf3, sf3)
            nc.vector.tensor_sub(aa3, aa3, epi3[:, :, 1 + OUT_CH:ch])

            out_sb = ep.tile([P, nt * OUT_CH], f32)
            out3 = out_sb.rearrange("p (t c) -> p t c", c=OUT_CH)
            for t in range(nt):
                nc.vector.tensor_scalar_mul(
                    out3[:, t, :], xw2_nat3[:, t, :], d2[:, t:t + 1]
                )
                nc.vector.tensor_sub(out3[:, t, :], aa3[:, t, :], out3[:, t, :])
                nc.vector.tensor_scalar_mul(
                    out3[:, t, :], out3[:, t, :], nrm[:, t:t + 1]
                )
                nc.vector.tensor_add(out3[:, t, :], out3[:, t, :], bias_sb)

            nc.gpsimd.dma_start(out_d.rearrange("(t p) c -> p t c", p=P), out3)

    nc.compile()
    return nc


def _get_nc(n_nodes: int, n_cores: int, variant: str | None = None):
    variant = variant or VARIANT
    key = (n_nodes, n_cores, variant)
    if key not in _BUILD_CACHE:
        _BUILD_CACHE[key] = _build(n_nodes, n_cores, variant)
    return _BUILD_CACHE[key]


def kernel(x, edge_index, weight, bias, n_cores: int = N_CORES,
           variant: str | None = None, trace: bool = False):
    from concourse import bass_utils

    variant = variant or VARIANT
    x = np.asarray(x, dtype=np.float32)
    edge_index = np.asarray(edge_index, dtype=np.float32)
    weight = np.asarray(weight, dtype=np.float32)
    bias = np.asarray(bias, dtype=np.float32)
    n = edge_index.shape[0]
    rpc = n // n_cores

    nc = _get_nc(n, n_cores, variant)

    # Host-side shard/packing: row-shard edge (the fp8/bf16 cast is lossless
    # for the 0/1 adjacency values), slice x rows, extract the diagonal shard.
    if variant == "fp8dr":
        edge_packed = edge_index.astype(ml_dtypes.float8_e4m3).view(np.uint16)
    else:
        edge_packed = edge_index.astype(ml_dtypes.bfloat16)
    dg = np.ascontiguousarray(np.diagonal(edge_index)).astype(np.float32)
    bias_rep = np.tile(bias[None, :], (P, 1)).astype(np.float32)
    ident = np.eye(P, dtype=np.float32)

    in_maps = []
    for c in range(n_cores):
        i0 = c * rpc
        in_maps.append({
            "edge": np.ascontiguousarray(edge_packed[i0:i0 + rpc]),
            "xfull": x,
            "x_rows": np.ascontiguousarray(x[i0:i0 + rpc]),
            "weight": weight,
            "bias_rep": bias_rep,
            "diag": np.ascontiguousarray(dg[i0:i0 + rpc]),
            "ident": ident,
        })

    res = bass_utils.run_bass_kernel_spmd(
        nc, in_maps, core_ids=list(range(n_cores)), trace=trace
    )
    out = np.concatenate([r["out"] for r in res.results], axis=0)
    kernel.last_results = res
    return out
